# revision 2
# baseline (speedup 1.0000x reference)
"""Multi-head causal attention on 8 Trainium2 NeuronCores.

Sharding: tensor-parallel over heads x data-parallel over batch.
Core c handles batch c//4 and heads [4*(c%4), 4*(c%4)+4). Each core
computes Q/K/V projections for its head slice over the full sequence,
causal flash-style attention (transposed scores, ones-column softmax
denominator), and a partial output projection against its row-slice of
W_o. The 4 partial outputs per batch are summed on the host (the
all-reduce of row-parallel W_o), which also adds b_o.
"""
import sys

sys.path.insert(0, '/opt/trn_rl_repo')

import numpy as np
import ml_dtypes

B, S, D, H, DK = 2, 2048, 1024, 16, 64
NCORES = 8
HL = 4            # heads per core
DL = HL * DK      # head-dim slice per core (256)
NQB = S // 512    # 512-wide query blocks
NKST = S // 128   # 128-wide key tiles

_cache = {}


def _build(repeat=1, dynamic=False, stage=4):
    """stage: 1=DMAs only, 2=+QKV projections, 3=+attention, 4=full."""
    import concourse.bacc as bacc
    import concourse.mybir as mybir
    import concourse.tile as tile
    from contextlib import ExitStack, nullcontext

    f32, f32r, bf16 = mybir.dt.float32, mybir.dt.float32r, mybir.dt.bfloat16
    Ident, Exp = mybir.ActivationFunctionType.Identity, mybir.ActivationFunctionType.Exp

    nc = bacc.Bacc("TRN2", target_bir_lowering=False, debug=False, num_devices=NCORES)
    xt_d = nc.dram_tensor("xt", (D, S), bf16, kind="ExternalInput").ap()
    wq_d = nc.dram_tensor("wq", (D, DL), bf16, kind="ExternalInput").ap()
    wk_d = nc.dram_tensor("wk", (D, DL), bf16, kind="ExternalInput").ap()
    wv_d = nc.dram_tensor("wv", (D, DL), bf16, kind="ExternalInput").ap()
    wo_d = nc.dram_tensor("wo", (DL, D), bf16, kind="ExternalInput").ap()
    bqk_d = nc.dram_tensor("bqk", (DL, 2), f32, kind="ExternalInput").ap()
    bv_d = nc.dram_tensor("bv", (DL,), f32, kind="ExternalInput").ap()
    mask_d = nc.dram_tensor("masks", (4, 128, 512), bf16, kind="ExternalInput").ap()
    po_d = nc.dram_tensor("po", (128, NQB, 8, 512), bf16, kind="ExternalOutput").ap()

    with tile.TileContext(nc) as tc:
        with ExitStack() as ctx:
            sb = ctx.enter_context(tc.tile_pool(name="sb", bufs=1))
            ps = ctx.enter_context(tc.tile_pool(name="ps", bufs=1, space="PSUM"))

            # ---- persistent SBUF tiles ----
            xt = sb.tile([128, 8, S], bf16, name="xt")
            wq_s = sb.tile([128, 8, DL], bf16, name="wq_s")
            wk_s = sb.tile([128, 8, DL], bf16, name="wk_s")
            wv_s = sb.tile([128, 8, DL], bf16, name="wv_s")
            wo_s = sb.tile([128, 2, D], bf16, name="wo_s")
            qt = [sb.tile([128, S], bf16, name=f"qt{p}") for p in range(2)]
            kt = [sb.tile([128, S], bf16, name=f"kt{p}") for p in range(2)]
            ctxt = [sb.tile([128, S], bf16, name=f"ctxt{p}") for p in range(2)]
            # v_aug: [128, ks-tile, 2 pairs x (64 even | one | 64 odd | spare)]
            v_aug = sb.tile([128, NKST, 264], bf16, name="v_aug")
            masks = sb.tile([128, 4, 512], bf16, name="masks")
            bqk_t = sb.tile([128, 2, 2], f32, name="bqk_t")
            bv_sb = sb.tile([1, DL], f32, name="bv_sb")
            bvB = sb.tile([128, DL], f32, name="bvB")

            rep_ctx = tc.For_i(0, repeat, 1) if dynamic else nullcontext(range(repeat))
            with rep_ctx as _it:
              for _rep in ([0] if dynamic else _it):
                # ---- input DMAs: sync ring: xt; scalar ring: weights; gpsimd: small ----
                xt_r = xt_d.rearrange("(k p) s -> p k s", p=128)
                nc.sync.dma_start(xt[:, 0:4, :], xt_r[:, 0:4, :])
                nc.sync.dma_start(xt[:, 4:8, :], xt_r[:, 4:8, :])
                nc.scalar.dma_start(bqk_t[:], bqk_d.rearrange("(p2 p) j -> p p2 j", p=128))
                nc.scalar.dma_start(wk_s[:], wk_d.rearrange("(k p) n -> p k n", p=128))
                nc.scalar.dma_start(wq_s[:], wq_d.rearrange("(k p) n -> p k n", p=128))
                nc.scalar.dma_start(wv_s[:], wv_d.rearrange("(k p) n -> p k n", p=128))
                nc.scalar.dma_start(wo_s[:], wo_d.rearrange("(k p) n -> p k n", p=128))
                nc.gpsimd.dma_start(masks[:], mask_d.rearrange("t p n -> p t n"))
                nc.gpsimd.dma_start(bv_sb[:], bv_d.rearrange("(o n) -> o n", o=1))
                nc.gpsimd.partition_broadcast(bvB[:], bv_sb[:])

                # ---- fused V-proj / attention / output projection, qb-major ----
                # ---- K / Q projections (phase) ----
                for dst, w_s, b_t in (((kt, wk_s, 1), (qt, wq_s, 0)) if stage >= 2 else ()):
                    for p in range(2):
                        for qb in range(NQB):
                            pp = ps.tile([128, 512], f32, tag="b512", bufs=2)
                            for k in range(8):
                                nc.tensor.matmul(pp[:], w_s[:, k, p * 128:(p + 1) * 128],
                                                 xt[:, k, qb * 512:(qb + 1) * 512],
                                                 start=(k == 0), stop=(k == 7))
                            nc.vector.tensor_scalar_add(dst[p][:, qb * 512:(qb + 1) * 512], pp[:],
                                                        bqk_t[:, p, b_t:b_t + 1])

                # ---- V projection (phase) ----
                for sp in range(NKST if stage >= 2 else 0):
                        pv = ps.tile([128, 512], f32, tag="b512", bufs=2)
                        for k in range(8):
                            nc.tensor.matmul(pv[:, 0:DL], xt[:, k, sp * 128:(sp + 1) * 128],
                                             wv_s[:, k, :], start=(k == 0), stop=(k == 7))
                        vdst = v_aug[:, sp, :].rearrange("p (pr e q) -> p pr e q", pr=2, e=2, q=66)
                        nc.vector.tensor_add(vdst[:, :, :, 0:64],
                                             pv[:, 0:DL].rearrange("p (pr e q) -> p pr e q", pr=2, e=2, q=64),
                                             bvB[:].rearrange("p (pr e q) -> p pr e q", pr=2, e=2, q=64))
                        nc.gpsimd.memset(vdst[:, :, :, 64:65], 1.0)

                # ---- attention (phase) ----
                for p in range(2 if stage >= 3 else 0):
                    for qb in range(NQB):
                        n_kst = 4 * qb + 4
                        n_grp = n_kst // 2
                        av0 = ps.tile([65, 512], f32, tag="b512", bufs=2)
                        av1 = ps.tile([65, 512], f32, tag="b512", bufs=2)
                        for g in range(n_grp):
                            sc0 = ps.tile([128, 1024], f32, tag="sc", bufs=3)
                            sc1 = ps.tile([128, 1024], f32, tag="sc", bufs=3)
                            for i in range(2):
                                kst = 2 * g + i
                                nc.tensor.matmul(sc0[:, i * 512:(i + 1) * 512],
                                                 kt[p][0:64, kst * 128:(kst + 1) * 128],
                                                 qt[p][0:64, qb * 512:(qb + 1) * 512],
                                                 start=True, stop=True, tile_position=(0, 0))
                                nc.tensor.matmul(sc1[:, i * 512:(i + 1) * 512],
                                                 kt[p][64:128, kst * 128:(kst + 1) * 128],
                                                 qt[p][64:128, qb * 512:(qb + 1) * 512],
                                                 start=True, stop=True, tile_position=(64, 0))
                            e0 = sb.tile([128, 1024], bf16, tag="ex", bufs=4)
                            e1 = sb.tile([128, 1024], bf16, tag="ex", bufs=4)
                            nc.scalar.activation(e0[:], sc0[:], Exp, scale=0.125)
                            nc.scalar.activation(e1[:], sc1[:], Exp, scale=0.125)
                            for i in range(2):
                                kst = 2 * g + i
                                mi = kst - 4 * qb
                                if mi >= 0:
                                    sl = slice(i * 512, (i + 1) * 512)
                                    nc.vector.tensor_mul(e0[:, sl], e0[:, sl], masks[:, mi, :])
                                    nc.vector.tensor_mul(e1[:, sl], e1[:, sl], masks[:, mi, :])
                            for i in range(2):
                                kst = 2 * g + i
                                st, sp_ = (g == 0 and i == 0), (g == n_grp - 1 and i == 1)
                                nc.tensor.matmul(av0[:], v_aug[:, kst, p * 132:p * 132 + 65],
                                                 e0[:, i * 512:(i + 1) * 512], start=st, stop=sp_)
                                nc.tensor.matmul(av1[:], v_aug[:, kst, p * 132 + 66:p * 132 + 131],
                                                 e1[:, i * 512:(i + 1) * 512], start=st, stop=sp_)
                        for e, av in ((0, av0), (1, av1)):
                            rc = sb.tile([1, 512], f32, tag="rc", bufs=2)
                            rb = sb.tile([64, 512], f32, tag="rb", bufs=2)
                            nc.vector.reciprocal(rc[:], av[64:65, :])
                            nc.gpsimd.partition_broadcast(rb[:], rc[:])
                            nc.vector.tensor_mul(ctxt[p][e * 64:(e + 1) * 64, qb * 512:(qb + 1) * 512],
                                                 av[0:64, :], rb[:])

                # ---- output projection (phase) ----
                for qb in range(NQB if stage >= 4 else 0):
                    po_sb = sb.tile([128, 8, 512], bf16, tag="po_s", bufs=2)
                    for ot in range(8):
                        po_p = ps.tile([128, 512], f32, tag="b512", bufs=2)
                        for k in range(2):
                            nc.tensor.matmul(po_p[:], wo_s[:, k, ot * 128:(ot + 1) * 128],
                                             ctxt[k][:, qb * 512:(qb + 1) * 512],
                                             start=(k == 0), stop=(k == 1))
                        if ot % 2 == 0:
                            nc.scalar.activation(po_sb[:, ot, :], po_p[:], Ident)
                        else:
                            nc.vector.tensor_copy(po_sb[:, ot, :], po_p[:])
                    nc.sync.dma_start(po_d[:, qb, :, :], po_sb[:])

    nc.compile()
    return nc


def _causal_mask_ok(mask):
    m = np.asarray(mask)
    if m.shape != (S, S):
        return False
    return np.array_equal(m.astype(bool), np.triu(np.ones((S, S), bool), k=1))


def _numpy_fallback(x, mask, Wq, bq, Wk, bk, Wv, bv, Wo, bo):
    x = np.asarray(x, np.float64)
    q = (x @ Wq + bq).reshape(B, S, H, DK).transpose(0, 2, 1, 3)
    k = (x @ Wk + bk).reshape(B, S, H, DK).transpose(0, 2, 1, 3)
    v = (x @ Wv + bv).reshape(B, S, H, DK).transpose(0, 2, 1, 3)
    s = np.einsum("bhqd,bhkd->bhqk", q, k) / np.sqrt(DK)
    s = np.where(np.asarray(mask, bool), -np.inf, s)
    s = s - s.max(-1, keepdims=True)
    e = np.exp(s)
    a = e / e.sum(-1, keepdims=True)
    ctx = np.einsum("bhqk,bhkd->bhqd", a, v).transpose(0, 2, 1, 3).reshape(B, S, D)
    return (ctx @ Wo + bo).astype(np.float32)


def _tri_masks():
    m = np.zeros((4, 128, 512), np.float32)
    n = np.arange(512)
    for t in range(4):
        for p_ in range(128):
            m[t, p_, :] = (n >= t * 128 + p_)
    return m.astype(ml_dtypes.bfloat16)


def _make_in_maps(x, Wq, bq, Wk, bk, Wv, bv, Wo):
    Wq, Wk, Wv, Wo = (np.asarray(w, np.float32) for w in (Wq, Wk, Wv, Wo))
    bq, bk, bv = (np.asarray(b_, np.float32) for b_ in (bq, bk, bv))
    masks_np = _tri_masks()
    xts = [np.ascontiguousarray(x[b_].T.astype(ml_dtypes.bfloat16)) for b_ in range(B)]

    in_maps = []
    for c in range(NCORES):
        b_, hs = c // 4, (c % 4) * DL
        in_maps.append({
            "xt": xts[b_],
            "wq": np.ascontiguousarray(Wq[:, hs:hs + DL].astype(ml_dtypes.bfloat16)),
            "wk": np.ascontiguousarray(Wk[:, hs:hs + DL].astype(ml_dtypes.bfloat16)),
            "wv": np.ascontiguousarray(Wv[:, hs:hs + DL].astype(ml_dtypes.bfloat16)),
            "wo": np.ascontiguousarray(Wo[hs:hs + DL, :].astype(ml_dtypes.bfloat16)),
            "bqk": np.ascontiguousarray(np.stack([bq[hs:hs + DL], bk[hs:hs + DL]], 1)),
            "bv": np.ascontiguousarray(bv[hs:hs + DL]),
            "masks": masks_np,
        })
    return in_maps


def kernel(x, mask, Wq, bq, Wk, bk, Wv, bv, Wo, bo):
    x = np.ascontiguousarray(np.asarray(x, np.float32))
    if not _causal_mask_ok(mask):
        return _numpy_fallback(x, mask, Wq, bq, Wk, bk, Wv, bv, Wo, bo)

    from concourse import bass_utils

    if "nc" not in _cache:
        _cache["nc"] = _build(repeat=1)
    nc = _cache["nc"]

    bo = np.asarray(bo, np.float32)
    in_maps = _make_in_maps(x, Wq, bq, Wk, bk, Wv, bv, Wo)

    res = bass_utils.run_bass_kernel_spmd(nc, in_maps, core_ids=list(range(NCORES)))

    out = np.empty((B, S, D), np.float32)
    for b_ in range(B):
        acc = res.results[b_ * 4]["po"].astype(np.float32)
        for g in range(1, 4):
            acc = acc + res.results[b_ * 4 + g]["po"]
        # acc[p, qb, k, s] = outT[k*128+p, qb*512+s]
        out[b_] = acc.transpose(1, 3, 2, 0).reshape(S, D) + bo
    return out



# revision 27
# speedup vs baseline: 1.5703x; 1.5703x over previous
"""Multi-head causal attention on 8 Trainium2 NeuronCores.

Sharding: tensor-parallel over heads x data-parallel over batch.
Core c handles batch c//4 and heads [4*(c%4), 4*(c%4)+4). Each core
computes Q/K/V projections for its head slice over the full sequence,
causal attention (transposed scores, ones-column softmax denominator),
and a partial output projection against its row-slice of W_o. The 4
partial outputs per batch are summed on the host (the all-reduce of
row-parallel W_o), which also adds b_o.

Schedule: fused single-pass pipeline. The K projection (and first two V
tiles) run k-chunk-outer across all 8 PSUM banks so the PE overlaps the
chunked xt input DMA; attention runs qb-major (256-wide query blocks)
with both heads of a pair sharing one 2-bank PSUM score tile (one exp
per group covers 1024 columns); remaining V/Q projections and the
output projection are issued as PE filler inside the Act-bound
attention groups.
"""
import sys

sys.path.insert(0, '/opt/trn_rl_repo')

from collections import deque

import numpy as np
import ml_dtypes

B, S, D, H, DK = 2, 2048, 1024, 16, 64
NCORES = 8
HL = 4            # heads per core
DL = HL * DK      # head-dim slice per core (256)
QB = 256          # attention query-block width
NQB = S // QB     # 8
NKST = S // 128   # 16 key tiles of 128

_cache = {}


def _build(repeat=1, dynamic=False, stage=4, exp_ident=False, mask_off=False, av_off=False, av_nostack=False, av64=False, exp2x=False, dve2x=False, norm_off=False, mm_bcast=False, o_drain='dve', o_solo=False, o_tail=True, o_between=False, pool2x=False):
    """stage: 1=DMAs only, 2=+projections, 3=+attention, 4=full."""
    import concourse.bacc as bacc
    import concourse.mybir as mybir
    import concourse.tile as tile
    from contextlib import ExitStack, nullcontext

    f32, bf16 = mybir.dt.float32, mybir.dt.bfloat16
    Exp = (mybir.ActivationFunctionType.Identity if exp_ident else
           mybir.ActivationFunctionType.Exp)

    nc = bacc.Bacc("TRN2", target_bir_lowering=False, debug=False, num_devices=NCORES)
    xt_d = nc.dram_tensor("xt", (D, S), bf16, kind="ExternalInput").ap()
    wq_d = nc.dram_tensor("wq", (D, DL), bf16, kind="ExternalInput").ap()
    wk_d = nc.dram_tensor("wk", (D, DL), bf16, kind="ExternalInput").ap()
    wv_d = nc.dram_tensor("wv", (D, DL), bf16, kind="ExternalInput").ap()
    wo_d = nc.dram_tensor("wo", (DL, D), bf16, kind="ExternalInput").ap()
    bqk_d = nc.dram_tensor("bqk", (DL, 2), f32, kind="ExternalInput").ap()
    bv_d = nc.dram_tensor("bv", (DL,), f32, kind="ExternalInput").ap()
    mask_d = nc.dram_tensor("masks", (2, 128, QB), bf16, kind="ExternalInput").ap()
    po_d = nc.dram_tensor("po", (128, NQB, 8, QB), bf16, kind="ExternalOutput").ap()

    with tile.TileContext(nc) as tc:
        with ExitStack() as ctx:
            sb = ctx.enter_context(tc.tile_pool(name="sb", bufs=1))
            ps = ctx.enter_context(tc.tile_pool(name="ps", bufs=1, space="PSUM"))

            # ---- persistent SBUF tiles ----
            xt = sb.tile([128, 8, S], bf16, name="xt")
            wq_s = sb.tile([128, 8, DL], bf16, name="wq_s")
            wk_s = sb.tile([128, 8, DL], bf16, name="wk_s")
            wv_s = sb.tile([128, 8, DL], bf16, name="wv_s")
            wo_s = sb.tile([128, 2, D], bf16, name="wo_s")
            qt = [sb.tile([128, S], bf16, name=f"qt{p}") for p in range(2)]
            kt = [sb.tile([128, S], bf16, name=f"kt{p}") for p in range(2)]
            ctxt = [sb.tile([128, S], bf16, name=f"ctxt{p}") for p in range(2)]
            # v_aug: [128, ks-tile, 2 pairs x 2 heads x (64 | one | spare)]
            v_aug = sb.tile([128, NKST, 264], bf16, name="v_aug")
            masks = sb.tile([128, 2, QB], bf16, name="masks")
            bqk_t = sb.tile([128, 2, 2], f32, name="bqk_t")
            bv_sb = sb.tile([1, DL], f32, name="bv_sb")
            bvB = sb.tile([128, DL], f32, name="bvB")
            ones64 = sb.tile([1, 64], bf16, name="ones64")
            nc.gpsimd.memset(ones64[:], 1.0)

            def v_drain(sp, pv):
                vdst = v_aug[:, sp, :].rearrange("p (pr e q) -> p pr e q",
                                                 pr=2, e=2, q=66)
                nc.vector.tensor_add(
                    vdst[:, :, :, 0:64],
                    pv[:, 0:DL].rearrange("p (pr e q) -> p pr e q", pr=2, e=2, q=64),
                    bvB[:].rearrange("p (pr e q) -> p pr e q", pr=2, e=2, q=64))
                nc.gpsimd.memset(vdst[:, :, :, 64:65], 1.0)

            rep_ctx = tc.For_i(0, repeat, 1) if dynamic else nullcontext(range(repeat))
            with rep_ctx as _it:
              for _rep in ([0] if dynamic else _it):
                # ---- input DMAs: xt split across sync+vector rings (parallel
                # transfer, subtile deps let K-proj consume k-chunks in order);
                # weights on scalar ring, wk/wv first (needed by startup pass) ----
                xt_r = xt_d.rearrange("(k p) s -> p k s", p=128)
                nc.scalar.dma_start(wk_s[:], wk_d.rearrange("(k p) n -> p k n", p=128))
                nc.sync.dma_start(xt[:, 0:4, :], xt_r[:, 0:4, :])
                nc.gpsimd.dma_start(xt[:, 4:8, :], xt_r[:, 4:8, :])
                nc.scalar.dma_start(wv_s[:], wv_d.rearrange("(k p) n -> p k n", p=128))
                nc.scalar.dma_start(wq_s[:], wq_d.rearrange("(k p) n -> p k n", p=128))
                nc.scalar.dma_start(wo_s[:], wo_d.rearrange("(k p) n -> p k n", p=128))
                nc.gpsimd.dma_start(bqk_t[:], bqk_d.rearrange("(p2 p) j -> p p2 j", p=128))
                nc.gpsimd.dma_start(masks[:], mask_d.rearrange("t p n -> p t n"))
                nc.gpsimd.dma_start(bv_sb[:], bv_d.rearrange("(o n) -> o n", o=1))
                nc.gpsimd.partition_broadcast(bvB[:], bv_sb[:])

                if stage < 2:
                    continue

                # ---- startup: K proj (6 blocks) + V sp0/sp1, k-chunk-outer
                # across all 8 PSUM banks, overlapping the chunked xt DMA ----
                sc_st = [ps.tile([128, 2, 2, QB], f32, tag="sc", bufs=2, name=f"scst{t}")
                         for t in range(2)]
                av_st = [ps.tile([128, 2, QB], f32, tag="av", bufs=2, name=f"avst{t}")
                         for t in range(2)]
                # 6 K accumulators: (pair, 512-block): b0, b1, b2
                kacc = [sc_st[0][:, 0, :, :], sc_st[0][:, 1, :, :],
                        sc_st[1][:, 0, :, :], sc_st[1][:, 1, :, :],
                        av_st[0][:, :, :], av_st[1][:, :, :]]
                kmap = [(0, 0), (1, 0), (0, 1), (1, 1), (0, 2), (1, 2)]
                vacc = [ps.tile([128, 512], f32, tag="b512", bufs=2, name=f"vst{t}")
                        for t in range(2)]
                for c in range(8):
                    for j, (p, b) in enumerate(kmap):
                        nc.tensor.matmul(kacc[j][:], wk_s[:, c, p * 128:(p + 1) * 128],
                                         xt[:, c, b * 512:(b + 1) * 512],
                                         start=(c == 0), stop=(c == 7))
                    for sp in range(2):
                        nc.tensor.matmul(vacc[sp][:, 0:DL],
                                         xt[:, c, sp * 128:(sp + 1) * 128],
                                         wv_s[:, c, :], start=(c == 0), stop=(c == 7))
                for j, (p, b) in enumerate(kmap):
                    nc.vector.tensor_scalar_add(kt[p][:, b * 512:(b + 1) * 512],
                                                kacc[j][:], bqk_t[:, p, 1:2])
                for sp in range(2):
                    v_drain(sp, vacc[sp])
                # leftover K blocks (p, b=3), k-inner
                for p in range(2):
                    kp = ps.tile([128, 512], f32, tag="b512", bufs=2, name="kp")
                    for c in range(8):
                        nc.tensor.matmul(kp[:], wk_s[:, c, p * 128:(p + 1) * 128],
                                         xt[:, c, 3 * 512:4 * 512],
                                         start=(c == 0), stop=(c == 7))
                    nc.vector.tensor_scalar_add(kt[p][:, 3 * 512:4 * 512], kp[:],
                                                bqk_t[:, p, 1:2])

                # ---- filler items ----
                def v_tile(sp):
                    def f():
                        pv = ps.tile([128, 512], f32, tag="b512", bufs=2, name="pv")
                        for c in range(8):
                            nc.tensor.matmul(pv[:, 0:DL], xt[:, c, sp * 128:(sp + 1) * 128],
                                             wv_s[:, c, :], start=(c == 0), stop=(c == 7))
                        v_drain(sp, pv)
                    return f

                def q_block(b):
                    def f():
                        for p in range(2):
                            qp = ps.tile([128, 512], f32, tag="b512", bufs=2, name="qp")
                            for c in range(8):
                                nc.tensor.matmul(qp[:], wq_s[:, c, p * 128:(p + 1) * 128],
                                                 xt[:, c, b * 512:(b + 1) * 512],
                                                 start=(c == 0), stop=(c == 7))
                            nc.vector.tensor_scalar_add(qt[p][:, b * 512:(b + 1) * 512],
                                                        qp[:], bqk_t[:, p, 0:1])
                    return f

                def o_tile(qb, ot, po_sb):
                    def f():
                        po_p = ps.tile([128, 512], f32, tag="b512", bufs=2, name="po_p")
                        for k in range(2):
                            nc.tensor.matmul(po_p[:, 0:QB],
                                             wo_s[:, k, ot * 128:(ot + 1) * 128],
                                             ctxt[k][:, qb * QB:(qb + 1) * QB],
                                             start=(k == 0), stop=(k == 1))
                        if o_drain == 'act' or (o_drain == 'alt' and ot % 2):
                            nc.scalar.copy(po_sb[:, ot, :], po_p[:, 0:QB])
                        else:
                            nc.vector.tensor_copy(po_sb[:, ot, :], po_p[:, 0:QB])
                    return f

                def o_dma(qb, po_sb):
                    def f():
                        nc.sync.dma_start(po_d[:, qb, :, :], po_sb[:])
                    return f

                # static filler order: V tiles with Q blocks interleaved at the
                # points they become prerequisites; o-items appended per qb.
                fillers = deque()
                for sp in range(2, 10):
                    fillers.append(('v', v_tile(sp)))
                    if sp == 3:
                        fillers.append(('q', q_block(1)))
                    if sp == 9:
                        fillers.append(('q', q_block(2)))
                for sp in range(10, NKST):
                    fillers.append(('v', v_tile(sp)))
                    if sp == 13:
                        fillers.append(('q', q_block(3)))
                counts = {'v': 2, 'q': 1, 'o': 0}

                def pop_one():
                    tag, f = fillers.popleft()
                    f()
                    counts[tag] += 1

                def ensure(v=0, q=0, o=0):
                    while counts['v'] < v or counts['q'] < q or counts['o'] < o:
                        pop_one()

                def run_one():
                    if fillers:
                        pop_one()

                # V sp0, sp1 done in startup; Q b0 explicit (needed by qb0)
                q_block(0)()

                if stage < 3:
                    while fillers:
                        pop_one()
                    if o_solo and stage >= 2:
                        for p in range(2):
                            nc.vector.memset(ctxt[p][:], 0.0)
                        for qb in range(NQB):
                            po_sb = sb.tile([128, 8, QB], bf16, tag="po_s", bufs=4,
                                            name="po_sb")
                            for ot in range(8):
                                o_tile(qb, ot, po_sb)()
                            o_dma(qb, po_sb)()
                    continue

                # ---- attention + interleaved fillers, qb-major ----
                tail_po = []
                pending_muls = []
                for qb in range(NQB):
                    # deps for this qb + don't outrun po_sb (bufs=4) reuse
                    ensure(v=2 * qb + 2, q=qb // 2 + 1,
                           o=9 * max(0, qb - 3)
                           if (stage >= 4 and not o_tail and not o_between) else 0)
                    po_sb = sb.tile([128, 8, QB], bf16, tag="po_s", bufs=4, name="po_sb")
                    for p in range(2):
                        av_t = ps.tile([128, 2, QB], f32, tag="av", bufs=2, name="av_t")

                        def av_issue(ep, gp):
                            if av_off:
                                return
                            for h in range(2):
                                for i in range(2):
                                    kst = 2 * gp + i
                                    nc.tensor.matmul(
                                        av_t[0:65, h, :],
                                        v_aug[:, kst, p * 132 + h * 66:p * 132 + h * 66 + 65],
                                        ep[:, h, i, :],
                                        start=(gp == 0 and i == 0 and h == 0),
                                        stop=(gp == qb and i == 1),
                                        skip_group_check=True)

                        pend = deque()  # exp tiles awaiting AV, depth 2 skew
                        for g in range(qb + 1):
                            sc = ps.tile([128, 2, 2, QB], f32, tag="sc", bufs=2, name="sc")
                            for h in range(2):
                                for i in range(2):
                                    kst = 2 * g + i
                                    nc.tensor.matmul(
                                        sc[:, h, i, :],
                                        kt[p][h * 64:(h + 1) * 64, kst * 128:(kst + 1) * 128],
                                        qt[p][h * 64:(h + 1) * 64, qb * QB:(qb + 1) * QB],
                                        start=(i == 0), stop=(i == 1),
                                        tile_position=(h * 64, 0),
                                        skip_group_check=True)
                            e = sb.tile([128, 2, 2, QB], bf16, tag="ex", bufs=4, name="e")
                            nc.scalar.activation(e[:], sc[:], Exp, scale=0.125)
                            if exp2x:
                                e2 = sb.tile([128, 2, 2, QB], bf16, tag="ex2", bufs=2, name="e2")
                                nc.scalar.activation(e2[:], e[:], Exp, scale=0.125)
                            if dve2x:
                                e3 = sb.tile([128, 2, 2, QB], bf16, tag="ex3", bufs=2, name="e3")
                                nc.vector.tensor_copy(e3[:], e[:])
                            if g == qb and not mask_off:  # diagonal masks
                                for h in range(2):
                                    nc.vector.tensor_mul(e[:, h, :, :], e[:, h, :, :],
                                                         masks[:])
                            pend.append((e, g))
                            if len(pend) > 2:
                                av_issue(*pend.popleft())
                            run_one()
                        while pend:
                            av_issue(*pend.popleft())
                            run_one()
                        if not (av_off or norm_off):
                            # issue the PREVIOUS pair's ctxt muls now (their
                            # broadcast has landed) so this pair's deferred
                            # muls never block the next pair's mask muls
                            for m in pending_muls:
                                m()
                            pending_muls = []
                            rc = sb.tile([1, 2, QB], f32, tag="rc", bufs=2, name="rc")
                            rb = sb.tile([64, 2, QB], f32, tag="rb", bufs=2, name="rb")
                            nc.vector.reciprocal(rc[:], av_t[64:65, :, :])
                            nc.gpsimd.partition_broadcast(rb[:], rc[:])

                            def mk_muls(av_t=av_t, rb=rb, p=p, qb=qb):
                                def f():
                                    for h in range(2):
                                        nc.vector.tensor_mul(
                                            ctxt[p][h * 64:(h + 1) * 64,
                                                    qb * QB:(qb + 1) * QB],
                                            av_t[0:64, h, :], rb[:, h, :])
                                return f
                            if pool2x:  # immediate-muls path
                                mk_muls()()
                            else:
                                pending_muls.append(mk_muls())
                        run_one()
                    if stage >= 4 and not o_tail:
                        for m in pending_muls:
                            m()
                        pending_muls = []
                    if stage >= 4 and o_between:
                        if qb > 0:
                            pqb, ppo = tail_po.pop()
                            for ot in range(8):
                                o_tile(pqb, ot, ppo)()
                            o_dma(pqb, ppo)()
                        tail_po.append((qb, po_sb))
                    elif stage >= 4 and not o_tail:
                        for ot in range(8):
                            fillers.append(('o', o_tile(qb, ot, po_sb)))
                        fillers.append(('o', o_dma(qb, po_sb)))
                    elif stage >= 4:
                        tail_po.append((qb, po_sb))

                for m in pending_muls:
                    m()
                pending_muls = []
                # drain remaining fillers (tail out-projections + DMAs)
                while fillers:
                    pop_one()
                for qb, po_sb in tail_po:
                    for ot in range(8):
                        o_tile(qb, ot, po_sb)()
                    o_dma(qb, po_sb)()

    nc.compile()
    return nc


def _causal_mask_ok(mask):
    m = np.asarray(mask)
    if m.shape != (S, S):
        return False
    return np.array_equal(m.astype(bool), np.triu(np.ones((S, S), bool), k=1))


def _numpy_fallback(x, mask, Wq, bq, Wk, bk, Wv, bv, Wo, bo):
    x = np.asarray(x, np.float64)
    q = (x @ Wq + bq).reshape(B, S, H, DK).transpose(0, 2, 1, 3)
    k = (x @ Wk + bk).reshape(B, S, H, DK).transpose(0, 2, 1, 3)
    v = (x @ Wv + bv).reshape(B, S, H, DK).transpose(0, 2, 1, 3)
    s = np.einsum("bhqd,bhkd->bhqk", q, k) / np.sqrt(DK)
    s = np.where(np.asarray(mask, bool), -np.inf, s)
    s = s - s.max(-1, keepdims=True)
    e = np.exp(s)
    a = e / e.sum(-1, keepdims=True)
    ctx = np.einsum("bhqk,bhkd->bhqd", a, v).transpose(0, 2, 1, 3).reshape(B, S, D)
    return (ctx @ Wo + bo).astype(np.float32)


def _masks256():
    m = np.zeros((2, 128, QB), np.float32)
    n = np.arange(QB)
    for t in range(2):
        for p_ in range(128):
            m[t, p_, :] = (n >= t * 128 + p_)
    return m.astype(ml_dtypes.bfloat16)


def _make_in_maps(x, Wq, bq, Wk, bk, Wv, bv, Wo):
    Wq, Wk, Wv, Wo = (np.asarray(w, np.float32) for w in (Wq, Wk, Wv, Wo))
    bq, bk, bv = (np.asarray(b_, np.float32) for b_ in (bq, bk, bv))
    masks_np = _masks256()
    xts = [np.ascontiguousarray(x[b_].T.astype(ml_dtypes.bfloat16)) for b_ in range(B)]

    in_maps = []
    for c in range(NCORES):
        b_, hs = c // 4, (c % 4) * DL
        in_maps.append({
            "xt": xts[b_],
            "wq": np.ascontiguousarray(Wq[:, hs:hs + DL].astype(ml_dtypes.bfloat16)),
            "wk": np.ascontiguousarray(Wk[:, hs:hs + DL].astype(ml_dtypes.bfloat16)),
            "wv": np.ascontiguousarray(Wv[:, hs:hs + DL].astype(ml_dtypes.bfloat16)),
            "wo": np.ascontiguousarray(Wo[hs:hs + DL, :].astype(ml_dtypes.bfloat16)),
            "bqk": np.ascontiguousarray(np.stack([bq[hs:hs + DL], bk[hs:hs + DL]], 1)),
            "bv": np.ascontiguousarray(bv[hs:hs + DL]),
            "masks": masks_np,
        })
    return in_maps


def kernel(x, mask, Wq, bq, Wk, bk, Wv, bv, Wo, bo):
    x = np.ascontiguousarray(np.asarray(x, np.float32))
    if not _causal_mask_ok(mask):
        return _numpy_fallback(x, mask, Wq, bq, Wk, bk, Wv, bv, Wo, bo)

    from concourse import bass_utils

    if "nc" not in _cache:
        _cache["nc"] = _build(repeat=1)
    nc = _cache["nc"]

    bo = np.asarray(bo, np.float32)
    in_maps = _make_in_maps(x, Wq, bq, Wk, bk, Wv, bv, Wo)

    res = bass_utils.run_bass_kernel_spmd(nc, in_maps, core_ids=list(range(NCORES)))

    out = np.empty((B, S, D), np.float32)
    for b_ in range(B):
        acc = res.results[b_ * 4]["po"].astype(np.float32)
        for g in range(1, 4):
            acc = acc + res.results[b_ * 4 + g]["po"]
        # acc[p, qb, k, n] = outT[k*128+p, qb*QB+n]
        out[b_] = acc.transpose(1, 3, 2, 0).reshape(S, D) + bo
    return out
